# revision 13
# baseline (speedup 1.0000x reference)
"""Trainium2 Bass kernel for an attention block.

Reference computation (per sample):
    xt  = x.T                             # [S, C]
    qkv = xt @ w_proj.T + b_proj          # [S, 3*H*dk], per head h: q|k|v
    attn = softmax(q @ k.T * dk**-0.5)    # [S, S] per head
    res  = (attn @ v) @ w_out.T + b_out + xt
    out  = res.T                          # [C, S]

Shapes: B=8, C=256, S=2048, H=4, dk=64.

Sharding: data-parallel over batch — each of the 8 NeuronCores computes one
full sample. No collectives.

Layout strategy (everything stays "transposed", channel-on-partition):
  - x_b is [C, S]: already the right orientation for every use (matmul rhs,
    matmul lhsT for the V projection, and the residual add).
  - qkT = W_qk @ x_b computed as [512, S] with head pairs packed per
    128-partition tile: QQ01 | KK01 | QQ23 | KK23 (weights pre-permuted on
    host).  Scores-transposed S^T[j,i] per head then needs no transposes:
    two row-tiled (tile_position) concurrent K=64 matmuls per j-tile, two
    heads writing the two halves of one [128, 1024] PSUM tile.
  - exp() runs as one wide ScalarE activation per [128, 1024] scores tile
    (scale=dk**-0.5 folded into the activation's free affine).
  - V is projected separately in S-layout ([S, 256]) with x_b as the
    stationary operand, stored per (j-tile, head) with an appended
    ones-column; the PV matmul then accumulates O_aug[65, 512] where row 64
    is the softmax denominator (flash-style augmented V).
  - Normalization: reciprocal of row 64, partition-broadcast via a K=1
    ones-matmul, one DVE multiply into attnT.
  - Output projection w_outT.T @ attnT lands directly in [C, S] orientation;
    residual-add x_b and DMA out.  No big-tensor transposes anywhere.

All matmuls use float32r (fp32 data, reduced-precision multiply) — 4x the
throughput of strict fp32 on the PE with far better accuracy than bf16.
The BIR verifier requires f32r matmul operands to come from an explicitly
rounding producer, so DMA-loaded fp32 gets a one-time DVE conversion copy and
all compute-produced operands use f32r output dtype directly.
"""

import os
import sys

for _p in ("/opt/trn_rl_repo", "/root/.axon_site/_ro/trn_rl_repo"):
    if os.path.isdir(_p) and _p not in sys.path:
        sys.path.append(_p)

import numpy as np

import concourse.bass as bass
import concourse.tile as tile
from concourse import mybir
from concourse.bass_utils import run_bass_kernel_spmd

N_CORES = 8
B, C, S = 8, 256, 2048
H, DK = 4, 64
SCALE = DK ** -0.5

F32 = mybir.dt.float32
F32R = mybir.dt.float32r
AFT = mybir.ActivationFunctionType


def _hoist_drain_waits(nc):
    """Hoist excess sem-waits off instructions onto EventSemaphores.

    The walrus build in this container encodes at most ONE sync wait per
    instruction (and none on Drain).  Excess waits become standalone
    EventSemaphore instructions immediately before the instruction on the
    same engine queue — identical stall semantics, just unfused.
    """
    for fn in nc.m.functions:
        for blk in fn.blocks:
            new_insts = []
            for inst in blk.instructions:
                si = inst.sync_info
                cap = 0 if isinstance(inst, mybir.InstDrain) else 1
                if si is not None and len(si.on_wait) > cap:
                    keep = si.on_wait[:cap]
                    hoist = si.on_wait[cap:]
                    for k, w in enumerate(hoist):
                        new_insts.append(
                            mybir.InstEventSemaphore(
                                name=f"{inst.name}-hoistwait{k}",
                                engine=inst.engine,
                                ins=[],
                                outs=[],
                                sync_info=mybir.SyncInfo(on_wait=[w], on_update=[]),
                            )
                        )
                    inst.sync_info = mybir.SyncInfo(
                        on_wait=list(keep), on_update=list(si.on_update)
                    )
                new_insts.append(inst)
            blk.instructions[:] = new_insts
    return nc


def build_nc(s=S, do_attn=True, do_out=True, wide_exp=True, do_pv=True):
    """Build the per-core Bass module. s = sequence length (param for sim)."""
    assert s % 512 == 0
    n_jt = s // 128          # j-tiles (keys)
    n_it = s // 512          # i-tiles (queries, 512 wide)
    n_nt = s // 512          # n-tiles for projections

    nc = bass.Bass("TRN2", target_bir_lowering=False, debug=False)
    xb_d = nc.dram_tensor("xb", [C, s], F32, kind="ExternalInput").ap()
    wqk_d = nc.dram_tensor("wqkT", [C, 512], F32, kind="ExternalInput").ap()
    wv_d = nc.dram_tensor("wvT", [C, 256], F32, kind="ExternalInput").ap()
    wo_d = nc.dram_tensor("woT", [C, C], F32, kind="ExternalInput").ap()
    bqk_d = nc.dram_tensor("bqk", [512], F32, kind="ExternalInput").ap()
    bv_d = nc.dram_tensor("bv", [256], F32, kind="ExternalInput").ap()
    bo_d = nc.dram_tensor("bo", [256], F32, kind="ExternalInput").ap()
    yb_d = nc.dram_tensor("yb", [C, s], F32, kind="ExternalOutput").ap()

    with tile.TileContext(nc) as tc:
        with (
            tc.tile_pool(name="singles", bufs=1) as singles,
            tc.tile_pool(name="pts", bufs=3) as pts,
            tc.tile_pool(name="recips", bufs=4) as recips,
            tc.tile_pool(name="outs", bufs=3) as outsb,
        ):
            # ---- resident SBUF tensors ----
            xsb = singles.tile([128, 2, s], F32)       # x_b k-tiles (residual)
            x_r = singles.tile([128, 2, s], F32R)      # x_b rounded for matmuls
            wqk_sb = singles.tile([128, 2, 512], F32)
            wqk_r = singles.tile([128, 2, 512], F32R)
            wv_sb = singles.tile([128, 2, 256], F32)
            wv_r = singles.tile([128, 2, 256], F32R)
            wo_sb = singles.tile([128, 2, 256], F32)
            wo_r = singles.tile([128, 2, 256], F32R)
            qk_sb = singles.tile([128, 4, s], F32R)    # QQ01 KK01 QQ23 KK23
            v_sb = singles.tile([128, n_jt, H, 65], F32R)  # V rows + ones col
            at_sb = singles.tile([128, 2, s], F32R)    # attnT (c' on partitions)
            bqk_sb = singles.tile([128, 4], F32)
            bo_sb = singles.tile([128, 2], F32)
            bv_sb = singles.tile([1, 256], F32)
            bv_r = singles.tile([1, 256], F32R)
            ones_f32 = singles.tile([1, 128], F32)
            ones_r = singles.tile([1, 128], F32R)
            ones64 = singles.tile([128, n_jt * H], F32)

            for kt in range(2):
                nc.sync.dma_start(out=xsb[:, kt, :], in_=xb_d[kt * 128:(kt + 1) * 128, :])
                nc.sync.dma_start(out=wqk_sb[:, kt, :], in_=wqk_d[kt * 128:(kt + 1) * 128, :])
                nc.sync.dma_start(out=wv_sb[:, kt, :], in_=wv_d[kt * 128:(kt + 1) * 128, :])
                nc.sync.dma_start(out=wo_sb[:, kt, :], in_=wo_d[kt * 128:(kt + 1) * 128, :])
                nc.vector.tensor_copy(x_r[:, kt, :], xsb[:, kt, :])
                nc.vector.tensor_copy(wqk_r[:, kt, :], wqk_sb[:, kt, :])
                nc.vector.tensor_copy(wv_r[:, kt, :], wv_sb[:, kt, :])
                nc.vector.tensor_copy(wo_r[:, kt, :], wo_sb[:, kt, :])
            nc.sync.dma_start(out=bqk_sb, in_=bqk_d.rearrange("(a b) -> b a", b=128))
            nc.sync.dma_start(out=bo_sb, in_=bo_d.rearrange("(a b) -> b a", b=128))
            nc.sync.dma_start(out=bv_sb, in_=bv_d[None, :])
            nc.vector.tensor_copy(bv_r, bv_sb)
            nc.vector.memset(ones_f32, 1.0)
            nc.vector.tensor_copy(ones_r, ones_f32)
            nc.vector.memset(ones64, 1.0)
            nc.vector.tensor_copy(
                v_sb[:, :, :, 64:65],
                ones64.rearrange("p (a b c) -> p a b c", a=n_jt, c=1),
            )

            # ---- phase 1: projections ----
            with tc.tile_pool(name="ps1", bufs=3, space="PSUM") as ps1:
                # Q/K projection: qkT[mt] = wqk[:, mt].T @ x  -> [128, s]
                for mt in range(4):
                    for nt in range(n_nt):
                        ps = ps1.tile([128, 512], F32, tag="pqk", name="pqk")
                        for kt in range(2):
                            nc.tensor.matmul(
                                ps,
                                wqk_r[:, kt, mt * 128:(mt + 1) * 128],
                                x_r[:, kt, nt * 512:(nt + 1) * 512],
                                start=(kt == 0),
                                stop=(kt == 1),
                            )
                        nc.vector.tensor_scalar_add(
                            qk_sb[:, mt, nt * 512:(nt + 1) * 512], ps,
                            bqk_sb[:, mt:mt + 1],
                        )
                # V projection in S-layout: v[st] = x[:, st].T @ wv + bv
                for st in range(n_jt):
                    ps = ps1.tile([128, 256], F32, tag="pv", name="pv")
                    for kt in range(2):
                        nc.tensor.matmul(
                            ps,
                            x_r[:, kt, st * 128:(st + 1) * 128],
                            wv_r[:, kt, :],
                            start=(kt == 0),
                            stop=False,
                        )
                    nc.tensor.matmul(
                        ps, ones_r, bv_r, start=False, stop=True,
                    )
                    nc.vector.tensor_copy(
                        v_sb[:, st, :, 0:64],
                        ps.rearrange("p (h c) -> p h c", h=H),
                    )

            # ---- phase 2: attention ----
            if not do_attn:
                do_out = False
            with (
                tc.tile_pool(name="ps_sc", bufs=2, space="PSUM") as ps_sc,
                tc.tile_pool(name="ps_o", bufs=1, space="PSUM") as ps_o,
                tc.tile_pool(name="ps_bc", bufs=1, space="PSUM") as ps_bc,
            ):
                for hp in range(2 if do_attn else 0):  # head pair
                    qq, kk = 2 * hp, 2 * hp + 1  # tile indices in qk_sb
                    for it in range(n_it):   # query tile, 512 wide
                        isl = slice(it * 512, (it + 1) * 512)
                        o_ps = [
                            ps_o.tile([128, 512], F32, tag=f"o{e}", name=f"o{e}")
                            for e in range(2)
                        ]
                        for jt in range(n_jt):
                            jsl = slice(jt * 128, (jt + 1) * 128)
                            sc = ps_sc.tile([128, 1024], F32, tag="sc", name="sc")
                            # scores^T for both heads, row-tiled concurrent
                            for e in range(2):  # e=0: head 2hp (parts 0:64)
                                psl = slice(64 * e, 64 * e + 64)
                                nc.tensor.matmul(
                                    sc[:, 512 * e:512 * e + 512],
                                    qk_sb[psl, kk, jsl],
                                    qk_sb[psl, qq, isl],
                                    start=True, stop=True,
                                )
                            pt = pts.tile([128, 1024], F32R, tag="pt", name="pt")
                            if wide_exp:
                                nc.scalar.activation(
                                    out=pt, in_=sc, func=AFT.Exp, scale=SCALE,
                                )
                            else:
                                for e in range(2):
                                    nc.scalar.activation(
                                        out=pt[:, 512 * e:512 * e + 512],
                                        in_=sc[:, 512 * e:512 * e + 512],
                                        func=AFT.Exp, scale=SCALE,
                                    )
                            if not do_pv:
                                continue
                            for e in range(2):
                                nc.tensor.matmul(
                                    o_ps[e][0:65, :],
                                    v_sb[:, jt, 2 * hp + e, :],
                                    pt[:, 512 * e:512 * e + 512],
                                    start=(jt == 0),
                                    stop=(jt == n_jt - 1),
                                )
                        # normalize: rows 0:64 /= row 64, write into attnT
                        for e in range(2 if do_pv else 0):
                            rec = recips.tile([1, 512], F32R, tag="rec", name="rec")
                            with nc.allow_low_precision(
                                reason="f32r rounding of softmax denominators"
                            ):
                                nc.vector.reciprocal(rec, o_ps[e][64:65, :])
                            bc = ps_bc.tile([64, 512], F32, tag="bc", name="bc")
                            nc.tensor.matmul(
                                bc, ones_r[:, 0:64], rec,
                                start=True, stop=True,
                            )
                            bcs = recips.tile([64, 512], F32, tag="bcs", name="bcs")
                            nc.vector.tensor_copy(bcs, bc)
                            nc.vector.tensor_mul(
                                at_sb[64 * e:64 * e + 64, hp, isl],
                                o_ps[e][0:64, :],
                                bcs,
                            )

            # ---- phase 3: output projection + residual ----
            with tc.tile_pool(name="ps2", bufs=3, space="PSUM") as ps2:
                for mt in range(2 if do_out else 0):
                    for nt in range(n_nt):
                        nsl = slice(nt * 512, (nt + 1) * 512)
                        ps = ps2.tile([128, 512], F32, tag="po", name="po")
                        for kt in range(2):
                            nc.tensor.matmul(
                                ps,
                                wo_r[:, kt, mt * 128:(mt + 1) * 128],
                                at_sb[:, kt, nsl],
                                start=(kt == 0),
                                stop=(kt == 1),
                            )
                        ot = outsb.tile([128, 512], F32, tag="ot", name="ot")
                        nc.vector.tensor_add(ot, ps, xsb[:, mt, nsl])
                        nc.vector.tensor_scalar_add(ot, ot, bo_sb[:, mt:mt + 1])
                        nc.sync.dma_start(
                            out=yb_d[mt * 128:(mt + 1) * 128, nsl], in_=ot,
                        )

    return nc


# Host-side weight permutation: pack heads for the row-tiled scores matmuls.
# w_proj rows (output channels), per head h (stride 3*DK=192):
#   q_h = rows [192h, 192h+64), k_h = +64, v_h = +128
def _perms():
    q = [np.arange(192 * h, 192 * h + 64) for h in range(H)]
    k = [np.arange(192 * h + 64, 192 * h + 128) for h in range(H)]
    v = [np.arange(192 * h + 128, 192 * h + 192) for h in range(H)]
    # QQ01 | KK01 | QQ23 | KK23
    perm_qk = np.concatenate([q[0], q[1], k[0], k[1], q[2], q[3], k[2], k[3]])
    perm_v = np.concatenate(v)
    return perm_qk, perm_v


_CACHE = {}


def _get_nc():
    if "nc" not in _CACHE:
        _CACHE["nc"] = _hoist_drain_waits(build_nc())
    return _CACHE["nc"]


def make_in_maps(x, w_proj, b_proj, w_out, b_out):
    perm_qk, perm_v = _perms()
    w_proj = np.asarray(w_proj, dtype=np.float32)
    wqkT = np.ascontiguousarray(w_proj[perm_qk, :].T)
    wvT = np.ascontiguousarray(w_proj[perm_v, :].T)
    woT = np.ascontiguousarray(np.asarray(w_out, dtype=np.float32).T)
    b_proj = np.asarray(b_proj, dtype=np.float32)
    bqk = np.ascontiguousarray(b_proj[perm_qk])
    bv = np.ascontiguousarray(b_proj[perm_v])
    bo = np.ascontiguousarray(np.asarray(b_out, dtype=np.float32))
    x = np.asarray(x, dtype=np.float32)
    return [
        {
            "xb": np.ascontiguousarray(x[b]),
            "wqkT": wqkT, "wvT": wvT, "woT": woT,
            "bqk": bqk, "bv": bv, "bo": bo,
        }
        for b in range(x.shape[0])
    ]


def kernel(x, w_proj, b_proj, w_out, b_out):
    x = np.asarray(x, dtype=np.float32)
    assert x.shape == (B, C, S), x.shape
    nc = _get_nc()
    in_maps = make_in_maps(x, w_proj, b_proj, w_out, b_out)
    res = run_bass_kernel_spmd(nc, in_maps, core_ids=list(range(N_CORES)))
    return np.stack([res.results[b]["yb"] for b in range(B)], axis=0)
